# revision 26
# baseline (speedup 1.0000x reference)
"""Distributed attention kernel for 8 TRN2 NeuronCores.

Sharding: core c -> (batch b = c // 4, head-group g = c % 4).
Each core computes, for its batch element, 4 of the 16 heads end-to-end
(QKV projection, rotary, attention, output projection), producing a
partial output for the full [S, D] result. The host sums the 4 group
partials per batch element (the "all-reduce after wo" done at unshard).

All layouts are pre-arranged on the host so the device does zero
transposes:
  - xT    [D, S]   : x[b].T                       (rhs for qk / lhsT for v)
  - wqk   [D, 1024]: q,k weight rows (rotary-pair-permuted) transposed
  - wv    [D, 512] : v weight rows transposed
  - wo    [512, D] : wo columns for this group, transposed
  - tabc  [128, S] : cos table doubled across both partition halves
  - tabs  [128, S] : sin table doubled (1/sqrt(hd) folded into wq)

Rotary trick: q/k weight rows are permuted per head so dims [0:64] are
the even (real) rotary components and [64:128] the odd (imag) ones.
Then rotary is plain elementwise math on partition halves. Scores are
invariant to this permutation since q and k share it.

Attention is computed transposed (scores^T[j, i]) so the softmax
numerator AND attn@v need no transposes; the softmax denominator comes
from a ones-column matmul riding the same exp stream, and the division
is applied to the raw attn@v output.
"""

import numpy as np
import ml_dtypes

import concourse.tile as tile
from concourse import bacc, mybir
from concourse.bass_utils import run_bass_kernel_spmd

B, S, D = 2, 2048, 2048
NH, HD = 16, 128
N_CORES = 8
GROUPS = 4
LH = NH // GROUPS  # 4 local heads
EQK = 2 * LH * HD  # 1024 (q chunks then k chunks)
EV = LH * HD  # 512
P = 128
DC = D // P  # 16 contraction chunks over d
SC = S // P  # 16 chunks over s
F = 512  # matmul moving free dim (1 PSUM bank of f32)
NT = S // F  # 4

CDT = mybir.dt.bfloat16
NP_CDT = ml_dtypes.bfloat16
F32 = mybir.dt.float32
NP_OUT = NP_CDT  # device out dtype (partials; host upcasts + sums)


def _make_psum_rr(mm_pool, acc_pool, l_pool):
    def gen():
        seq = (
            ("mm", mm_pool),
            ("acc", acc_pool),
            ("mm", mm_pool),
            ("lsum", l_pool),
            ("acc", acc_pool),
            ("lsum", l_pool),
        )
        i = 0
        while True:
            tag, pool = seq[i % len(seq)]
            i += 1
            t1bank = pool.tile([P, F], F32, tag=tag, name=f"rr{i}")
            yield t1bank

    return gen()


def build_graph(num_devices: int = N_CORES, reps: int = 1):
    """reps > 1 replicates the whole computation (timing instrumentation)."""
    nc = bacc.Bacc(
        "TRN2", target_bir_lowering=False, debug=False, num_devices=num_devices
    )
    xT = nc.dram_tensor("xT", [D, S], CDT, kind="ExternalInput").ap()
    wqk = nc.dram_tensor("wqk", [D, EQK], CDT, kind="ExternalInput").ap()
    wv = nc.dram_tensor("wv", [D, EV], CDT, kind="ExternalInput").ap()
    wo = nc.dram_tensor("wo", [EV, D], CDT, kind="ExternalInput").ap()
    tabc = nc.dram_tensor("tabc", [P, S], CDT, kind="ExternalInput").ap()
    tabs = nc.dram_tensor("tabs", [P, S], CDT, kind="ExternalInput").ap()
    out = nc.dram_tensor("out", [S, D], CDT, kind="ExternalOutput").ap()

    xT_r = xT.rearrange("(c p) s -> p c s", p=P)  # [128, 16, 2048]
    wqk_r = wqk.rearrange("(c p) e -> p c e", p=P)  # [128, 16, 1024]
    wv_r = wv.rearrange("(c p) e -> p c e", p=P)  # [128, 16, 512]
    wo_r = wo.rearrange("(c p) o -> p c o", p=P)  # [128, 4, 2048]
    out_r = out.rearrange("(c p) o -> c p o", p=P)  # [16, 128, 2048]

    Exp = mybir.ActivationFunctionType.Exp
    sub = mybir.AluOpType.subtract

    with tile.TileContext(nc) as tc:
        with (
            tc.tile_pool(name="big", bufs=1) as big,  # x slot, reused for wo
            tc.tile_pool(name="wqkp", bufs=1) as wqkp,
            tc.tile_pool(name="wvp", bufs=1) as wvp,  # wv slot, reused for attn
            tc.tile_pool(name="data", bufs=1) as data,
            tc.tile_pool(name="tmp", bufs=1) as tmpp,
            tc.tile_pool(name="expp", bufs=3) as expp,
            tc.tile_pool(name="small", bufs=2) as small,
            tc.tile_pool(name="ostage", bufs=6) as ostagep,
            tc.tile_pool(name="mm", bufs=3, space="PSUM") as mm_pool,
            tc.tile_pool(name="acc", bufs=3, space="PSUM") as acc_pool,
            tc.tile_pool(name="lsum", bufs=2, space="PSUM") as l_pool,
        ):
          for _rep in range(reps):
            # Round-robin 1-bank psum accumulators across all pools (the
            # attention-specific pools are idle outside attention) so the
            # PE can run several accumulation groups ahead of the
            # consumers. mm tiles are 2-bank [P, 2, F]; qk/v/wo use bank 0.
            rr = _make_psum_rr(mm_pool, acc_pool, l_pool)
            # ---------------- loads ----------------
            x_sb = big.tile([P, DC, S], CDT, tag="big")
            for c in range(DC):
                nc.sync.dma_start(x_sb[:, c, :], xT_r[:, c, :])
            wqk_sb = wqkp.tile([P, DC, EQK], CDT, tag="wqk")
            for c in range(DC):
                nc.sync.dma_start(wqk_sb[:, c, :], wqk_r[:, c, :])
            wv_sb = wvp.tile([P, DC, EV], CDT, tag="wv")
            for c in range(DC):
                nc.sync.dma_start(wv_sb[:, c, :], wv_r[:, c, :])
            tabc_sb = data.tile([P, S], CDT, tag="tabc")
            nc.sync.dma_start(tabc_sb[:], tabc)
            tabs_sb = data.tile([P, S], CDT, tag="tabs")
            nc.sync.dma_start(tabs_sb[:], tabs)

            rot_sb = data.tile([P, 2 * LH, S], CDT, tag="rot")
            v_sb = data.tile([P, SC, EV], CDT, tag="v")
            ones_sb = data.tile([P, P], CDT, tag="ones")
            nc.vector.memset(ones_sb[:], 1.0)

            # ---------------- qk projection + rotary ----------------
            # qkT[e, s] = sum_d wqk[d, e] * xT[d, s]; then rotary into rot_sb.
            for ec in range(2 * LH):
                for st in range(NT):
                    ps = next(rr)
                    for c in range(DC):
                        nc.tensor.matmul(
                            ps[:],
                            lhsT=wqk_sb[:, c, ec * P : (ec + 1) * P],
                            rhs=x_sb[:, c, st * F : (st + 1) * F],
                            start=(c == 0),
                            stop=(c == DC - 1),
                        )
                    sl = slice(st * F, (st + 1) * F)
                    # Stage psum -> bf16 SBUF on the (idle) scalar engine:
                    # qs straight, qsw with partition halves swapped. DVE
                    # then runs aligned-base bf16-SBUF ops in the 2x mode.
                    # partitions 0:64 = even (re), 64:128 = odd (im)
                    qs = tmpp.tile([P, F], CDT, tag="qs")
                    nc.scalar.copy(out=qs[:], in_=ps[:])
                    qsw = tmpp.tile([P, F], CDT, tag="qsw")
                    nc.scalar.copy(out=qsw[0:64], in_=ps[64:128])
                    nc.scalar.copy(out=qsw[64:128], in_=ps[0:64])
                    t1 = tmpp.tile([P, F], CDT, tag="t1")
                    t2 = tmpp.tile([P, F], CDT, tag="t2")
                    nc.vector.tensor_mul(t1[:], qs[:], tabc_sb[:, sl])
                    nc.vector.tensor_mul(t2[:], qsw[:], tabs_sb[:, sl])
                    nc.vector.tensor_tensor(
                        rot_sb[0:64, ec, sl], t1[0:64], t2[0:64], sub
                    )
                    nc.vector.tensor_add(
                        rot_sb[64:128, ec, sl], t1[64:128], t2[64:128]
                    )

            # ---------------- v projection ----------------
            # v[s, e] = sum_d xT[d, s] * wv[d, e]
            for sc in range(SC):
                ps = next(rr)
                for c in range(DC):
                    nc.tensor.matmul(
                        ps[:],
                        lhsT=x_sb[:, c, sc * P : (sc + 1) * P],
                        rhs=wv_sb[:, c, :],
                        start=(c == 0),
                        stop=(c == DC - 1),
                    )
                nc.any.tensor_copy(out=v_sb[:, sc, :], in_=ps[:])

            attn_sb = wvp.tile([P, LH, S], CDT, tag="wv")  # reuses wv slot

            # ---------------- attention (transposed scores) ----------------
            for h in range(LH):
                for it in range(NT):
                    isl = slice(it * F, (it + 1) * F)
                    po = acc_pool.tile([P, F], F32, tag="acc")
                    pl = l_pool.tile([P, F], F32, tag="lsum")
                    for jc in range(SC):
                        ps = mm_pool.tile([P, F], F32, tag="mm")
                        # scores^T[j, i] = sum_hd k[hd, j] * q[hd, i]
                        nc.tensor.matmul(
                            ps[:],
                            lhsT=rot_sb[:, LH + h, jc * P : (jc + 1) * P],
                            rhs=rot_sb[:, h, isl],
                            start=True,
                            stop=True,
                        )
                        et = expp.tile([P, F], CDT, tag="exp")
                        nc.scalar.activation(out=et[:], in_=ps[:], func=Exp)
                        nc.tensor.matmul(
                            pl[:],
                            lhsT=ones_sb[:, :],
                            rhs=et[:],
                            start=(jc == 0),
                            stop=(jc == SC - 1),
                        )
                        nc.tensor.matmul(
                            po[:],
                            lhsT=v_sb[:, jc, h * P : (h + 1) * P],
                            rhs=et[:],
                            start=(jc == 0),
                            stop=(jc == SC - 1),
                        )
                    # pl rows are all equal (ones lhsT) -> reciprocal is
                    # already "broadcast" across partitions.
                    rl128 = small.tile([P, F], F32, tag="recip128")
                    nc.vector.reciprocal(rl128[:], pl[:])
                    nc.vector.tensor_mul(attn_sb[:, h, isl], po[:], rl128[:])

            # ---------------- output projection ----------------
            wo_sb = big.tile([P, LH, D], CDT, tag="big")  # reuses x slot
            for c in range(LH):
                nc.sync.dma_start(wo_sb[:, c, :], wo_r[:, c, :])

            for sc in range(SC):
                for ot in range(NT):
                    osl = slice(ot * F, (ot + 1) * F)
                    pw = next(rr)
                    for hc in range(LH):
                        nc.tensor.matmul(
                            pw[:],
                            lhsT=attn_sb[:, hc, sc * P : (sc + 1) * P],
                            rhs=wo_sb[:, hc, osl],
                            start=(hc == 0),
                            stop=(hc == LH - 1),
                        )
                    ost = ostagep.tile([P, F], CDT, tag="ostage")
                    nc.any.tensor_copy(out=ost[:], in_=pw[:])
                    nc.sync.dma_start(out_r[sc, :, osl], ost[:])

    nc.compile()
    return nc


def shard_inputs(x, freqs_cis, wqkv, wo):
    """Produce the 8 per-core input maps (host-side layout prep)."""
    x = np.asarray(x, dtype=np.float32)
    freqs_cis = np.asarray(freqs_cis, dtype=np.float32)
    wqkv = np.asarray(wqkv, dtype=np.float32)
    wo = np.asarray(wo, dtype=np.float32)

    perm = np.concatenate([np.arange(0, HD, 2), np.arange(1, HD, 2)])  # even|odd
    cos = freqs_cis[:, :, 0].T  # [64, S]
    sin = freqs_cis[:, :, 1].T
    scale = 1.0 / np.sqrt(HD)  # folded into wq rows below
    tabc = np.concatenate([cos, cos], axis=0)  # [128, S]
    tabs = np.concatenate([sin, sin], axis=0)

    tabc = np.ascontiguousarray(tabc.astype(NP_CDT))
    tabs = np.ascontiguousarray(tabs.astype(NP_CDT))

    in_maps = []
    for c in range(N_CORES):
        b, g = divmod(c, GROUPS)
        heads = range(g * LH, (g + 1) * LH)
        wq_rows = np.concatenate(
            [wqkv[h * HD : (h + 1) * HD][perm] for h in heads], axis=0
        ) * scale  # [512, D]; 1/sqrt(hd) folded in
        wk_rows = np.concatenate(
            [wqkv[D + h * HD : D + (h + 1) * HD][perm] for h in heads], axis=0
        )
        wv_rows = np.concatenate(
            [wqkv[2 * D + h * HD : 2 * D + (h + 1) * HD] for h in heads], axis=0
        )
        wqk_l = np.concatenate([wq_rows, wk_rows], axis=0).T  # [D, 1024]
        wv_l = wv_rows.T  # [D, 512]
        din = np.concatenate([np.arange(h * HD, (h + 1) * HD) for h in heads])
        wo_l = wo[:, din].T  # [512, D]
        in_maps.append(
            {
                "xT": np.ascontiguousarray(x[b].T.astype(NP_CDT)),
                "wqk": np.ascontiguousarray(wqk_l.astype(NP_CDT)),
                "wv": np.ascontiguousarray(wv_l.astype(NP_CDT)),
                "wo": np.ascontiguousarray(wo_l.astype(NP_CDT)),
                "tabc": tabc,
                "tabs": tabs,
            }
        )
    return in_maps


def unshard_outputs(results):
    out = np.zeros((B, S, D), dtype=np.float32)
    for c in range(N_CORES):
        b = c // GROUPS
        out[b] += results[c]["out"].astype(np.float32)
    return out


_GRAPH_CACHE = {}


def kernel(x, freqs_cis, wqkv, wo):
    if "nc" not in _GRAPH_CACHE:
        _GRAPH_CACHE["nc"] = build_graph()
    nc = _GRAPH_CACHE["nc"]
    in_maps = shard_inputs(x, freqs_cis, wqkv, wo)
    res = run_bass_kernel_spmd(nc, in_maps, core_ids=list(range(N_CORES)))
    return unshard_outputs(res.results)


# revision 27
# speedup vs baseline: 1.0142x; 1.0142x over previous
"""Distributed attention kernel for 8 TRN2 NeuronCores.

Sharding: core c -> (batch b = c // 4, head-group g = c % 4).
Each core computes, for its batch element, 4 of the 16 heads end-to-end
(QKV projection, rotary, attention, output projection), producing a
partial output for the full [S, D] result. The host sums the 4 group
partials per batch element (the "all-reduce after wo" done at unshard).

All layouts are pre-arranged on the host so the device does zero
transposes:
  - xT    [D, S]   : x[b].T                       (rhs for qk / lhsT for v)
  - wqk   [D, 1024]: q,k weight rows (rotary-pair-permuted) transposed
  - wv    [D, 512] : v weight rows transposed
  - wo    [512, D] : wo columns for this group, transposed
  - tabc  [128, S] : cos table doubled across both partition halves
  - tabs  [128, S] : sin table doubled (1/sqrt(hd) folded into wq)

Rotary trick: q/k weight rows are permuted per head so dims [0:64] are
the even (real) rotary components and [64:128] the odd (imag) ones.
Then rotary is plain elementwise math on partition halves. Scores are
invariant to this permutation since q and k share it.

Attention is computed transposed (scores^T[j, i]) so the softmax
numerator AND attn@v need no transposes; the softmax denominator comes
from a ones-column matmul riding the same exp stream, and the division
is applied to the raw attn@v output.
"""

import numpy as np
import ml_dtypes

import concourse.tile as tile
from concourse import bacc, mybir
from concourse.bass_utils import run_bass_kernel_spmd

B, S, D = 2, 2048, 2048
NH, HD = 16, 128
N_CORES = 8
GROUPS = 4
LH = NH // GROUPS  # 4 local heads
EQK = 2 * LH * HD  # 1024 (q chunks then k chunks)
EV = LH * HD  # 512
P = 128
DC = D // P  # 16 contraction chunks over d
SC = S // P  # 16 chunks over s
F = 512  # matmul moving free dim (1 PSUM bank of f32)
NT = S // F  # 4

CDT = mybir.dt.bfloat16
NP_CDT = ml_dtypes.bfloat16
F32 = mybir.dt.float32
NP_OUT = NP_CDT  # device out dtype (partials; host upcasts + sums)


def _make_psum_rr(mm_pool, acc_pool, l_pool):
    def gen():
        seq = (
            ("mm", mm_pool),
            ("acc", acc_pool),
            ("mm", mm_pool),
            ("lsum", l_pool),
            ("acc", acc_pool),
            ("lsum", l_pool),
        )
        i = 0
        while True:
            tag, pool = seq[i % len(seq)]
            i += 1
            t1bank = pool.tile([P, F], F32, tag=tag, name=f"rr{i}")
            yield t1bank

    return gen()


def build_graph(num_devices: int = N_CORES, reps: int = 1):
    """reps > 1 replicates the whole computation (timing instrumentation)."""
    nc = bacc.Bacc(
        "TRN2", target_bir_lowering=False, debug=False, num_devices=num_devices
    )
    xT = nc.dram_tensor("xT", [D, S], CDT, kind="ExternalInput").ap()
    wqk = nc.dram_tensor("wqk", [D, EQK], CDT, kind="ExternalInput").ap()
    wv = nc.dram_tensor("wv", [D, EV], CDT, kind="ExternalInput").ap()
    wo = nc.dram_tensor("wo", [EV, D], CDT, kind="ExternalInput").ap()
    tabc = nc.dram_tensor("tabc", [P, S], CDT, kind="ExternalInput").ap()
    tabs = nc.dram_tensor("tabs", [P, S], CDT, kind="ExternalInput").ap()
    out = nc.dram_tensor("out", [S, D], CDT, kind="ExternalOutput").ap()

    xT_r = xT.rearrange("(c p) s -> p c s", p=P)  # [128, 16, 2048]
    wqk_r = wqk.rearrange("(c p) e -> p c e", p=P)  # [128, 16, 1024]
    wv_r = wv.rearrange("(c p) e -> p c e", p=P)  # [128, 16, 512]
    wo_r = wo.rearrange("(c p) o -> p c o", p=P)  # [128, 4, 2048]
    out_r = out.rearrange("(c p) o -> c p o", p=P)  # [16, 128, 2048]

    Exp = mybir.ActivationFunctionType.Exp
    sub = mybir.AluOpType.subtract

    with tile.TileContext(nc) as tc:
        with (
            tc.tile_pool(name="big", bufs=1) as big,  # x slot, reused for wo
            tc.tile_pool(name="wqkp", bufs=1) as wqkp,
            tc.tile_pool(name="wvp", bufs=1) as wvp,  # wv slot, reused for attn
            tc.tile_pool(name="data", bufs=1) as data,
            tc.tile_pool(name="tmp", bufs=1) as tmpp,
            tc.tile_pool(name="expp", bufs=3) as expp,
            tc.tile_pool(name="small", bufs=2) as small,
            tc.tile_pool(name="ostage", bufs=6) as ostagep,
            tc.tile_pool(name="mm", bufs=3, space="PSUM") as mm_pool,
            tc.tile_pool(name="acc", bufs=3, space="PSUM") as acc_pool,
            tc.tile_pool(name="lsum", bufs=2, space="PSUM") as l_pool,
        ):
          for _rep in range(reps):
            # Round-robin psum accumulators across all pools (the
            # attention-specific pools are idle outside attention) so the
            # PE can run several accumulation groups ahead of the consumers.
            rr = _make_psum_rr(mm_pool, acc_pool, l_pool)
            # ---------------- loads ----------------
            x_sb = big.tile([P, DC, S], CDT, tag="big")
            for c in range(DC):
                nc.sync.dma_start(x_sb[:, c, :], xT_r[:, c, :])
            wqk_sb = wqkp.tile([P, DC, EQK], CDT, tag="wqk")
            for c in range(DC):
                nc.sync.dma_start(wqk_sb[:, c, :], wqk_r[:, c, :])
            wv_sb = wvp.tile([P, DC, EV], CDT, tag="wv")
            for c in range(DC):
                nc.sync.dma_start(wv_sb[:, c, :], wv_r[:, c, :])
            tabc_sb = data.tile([P, S], CDT, tag="tabc")
            nc.sync.dma_start(tabc_sb[:], tabc)
            tabs_sb = data.tile([P, S], CDT, tag="tabs")
            nc.sync.dma_start(tabs_sb[:], tabs)

            rot_sb = data.tile([P, 2 * LH, S], CDT, tag="rot")
            v_sb = data.tile([P, SC, EV], CDT, tag="v")
            ones_sb = data.tile([P, P], CDT, tag="ones")
            nc.vector.memset(ones_sb[:], 1.0)

            # ---------------- qk projection + rotary ----------------
            # qkT[e, s] = sum_d wqk[d, e] * xT[d, s]; then rotary into rot_sb.
            for ec in range(2 * LH):
                for st in range(NT):
                    ps = next(rr)
                    for c in range(DC):
                        nc.tensor.matmul(
                            ps[:],
                            lhsT=wqk_sb[:, c, ec * P : (ec + 1) * P],
                            rhs=x_sb[:, c, st * F : (st + 1) * F],
                            start=(c == 0),
                            stop=(c == DC - 1),
                        )
                    sl = slice(st * F, (st + 1) * F)
                    # Stage psum -> bf16 SBUF on the (idle) scalar engine:
                    # qs straight, qsw with partition halves swapped. DVE
                    # then runs aligned-base bf16-SBUF ops in the 2x mode.
                    # partitions 0:64 = even (re), 64:128 = odd (im)
                    qs = tmpp.tile([P, F], CDT, tag="qs")
                    nc.scalar.copy(out=qs[:], in_=ps[:])
                    qsw = tmpp.tile([P, F], CDT, tag="qsw")
                    nc.scalar.copy(out=qsw[0:64], in_=ps[64:128])
                    nc.scalar.copy(out=qsw[64:128], in_=ps[0:64])
                    t1 = tmpp.tile([P, F], CDT, tag="t1")
                    t2 = tmpp.tile([P, F], CDT, tag="t2")
                    nc.vector.tensor_mul(t1[:], qs[:], tabc_sb[:, sl])
                    nc.vector.tensor_mul(t2[:], qsw[:], tabs_sb[:, sl])
                    nc.vector.tensor_tensor(
                        rot_sb[0:64, ec, sl], t1[0:64], t2[0:64], sub
                    )
                    nc.vector.tensor_add(
                        rot_sb[64:128, ec, sl], t1[64:128], t2[64:128]
                    )

            # ---------------- v projection ----------------
            # v[s, e] = sum_d xT[d, s] * wv[d, e]
            for sc in range(SC):
                ps = next(rr)
                for c in range(DC):
                    nc.tensor.matmul(
                        ps[:],
                        lhsT=x_sb[:, c, sc * P : (sc + 1) * P],
                        rhs=wv_sb[:, c, :],
                        start=(c == 0),
                        stop=(c == DC - 1),
                    )
                nc.any.tensor_copy(out=v_sb[:, sc, :], in_=ps[:])

            attn_sb = wvp.tile([P, LH, S], CDT, tag="wv")  # reuses wv slot

            # ---------------- attention (transposed scores) ----------------
            for h in range(LH):
                for it in range(NT):
                    isl = slice(it * F, (it + 1) * F)
                    po = acc_pool.tile([P, F], F32, tag="acc")
                    pl = l_pool.tile([P, F], F32, tag="lsum")
                    for jc in range(SC):
                        ps = mm_pool.tile([P, F], F32, tag="mm")
                        # scores^T[j, i] = sum_hd k[hd, j] * q[hd, i]
                        nc.tensor.matmul(
                            ps[:],
                            lhsT=rot_sb[:, LH + h, jc * P : (jc + 1) * P],
                            rhs=rot_sb[:, h, isl],
                            start=True,
                            stop=True,
                        )
                        et = expp.tile([P, F], CDT, tag="exp")
                        nc.scalar.activation(out=et[:], in_=ps[:], func=Exp)
                        nc.tensor.matmul(
                            pl[:],
                            lhsT=ones_sb[:, :],
                            rhs=et[:],
                            start=(jc == 0),
                            stop=(jc == SC - 1),
                        )
                        nc.tensor.matmul(
                            po[:],
                            lhsT=v_sb[:, jc, h * P : (h + 1) * P],
                            rhs=et[:],
                            start=(jc == 0),
                            stop=(jc == SC - 1),
                        )
                    # pl rows are all equal (ones lhsT) -> reciprocal is
                    # already "broadcast" across partitions.
                    rl128 = small.tile([P, F], F32, tag="recip128")
                    nc.vector.reciprocal(rl128[:], pl[:])
                    nc.vector.tensor_mul(attn_sb[:, h, isl], po[:], rl128[:])

            # ---------------- output projection ----------------
            wo_sb = big.tile([P, LH, D], CDT, tag="big")  # reuses x slot
            for c in range(LH):
                nc.sync.dma_start(wo_sb[:, c, :], wo_r[:, c, :])

            for sc in range(SC):
                for ot in range(NT):
                    osl = slice(ot * F, (ot + 1) * F)
                    pw = next(rr)
                    for hc in range(LH):
                        nc.tensor.matmul(
                            pw[:],
                            lhsT=attn_sb[:, hc, sc * P : (sc + 1) * P],
                            rhs=wo_sb[:, hc, osl],
                            start=(hc == 0),
                            stop=(hc == LH - 1),
                        )
                    ost = ostagep.tile([P, F], CDT, tag="ostage")
                    nc.any.tensor_copy(out=ost[:], in_=pw[:])
                    nc.sync.dma_start(out_r[sc, :, osl], ost[:])

    nc.compile()
    return nc


def shard_inputs(x, freqs_cis, wqkv, wo):
    """Produce the 8 per-core input maps (host-side layout prep)."""
    x = np.asarray(x, dtype=np.float32)
    freqs_cis = np.asarray(freqs_cis, dtype=np.float32)
    wqkv = np.asarray(wqkv, dtype=np.float32)
    wo = np.asarray(wo, dtype=np.float32)

    perm = np.concatenate([np.arange(0, HD, 2), np.arange(1, HD, 2)])  # even|odd
    cos = freqs_cis[:, :, 0].T  # [64, S]
    sin = freqs_cis[:, :, 1].T
    scale = 1.0 / np.sqrt(HD)  # folded into wq rows below
    tabc = np.concatenate([cos, cos], axis=0)  # [128, S]
    tabs = np.concatenate([sin, sin], axis=0)

    tabc = np.ascontiguousarray(tabc.astype(NP_CDT))
    tabs = np.ascontiguousarray(tabs.astype(NP_CDT))

    in_maps = []
    for c in range(N_CORES):
        b, g = divmod(c, GROUPS)
        heads = range(g * LH, (g + 1) * LH)
        wq_rows = np.concatenate(
            [wqkv[h * HD : (h + 1) * HD][perm] for h in heads], axis=0
        ) * scale  # [512, D]; 1/sqrt(hd) folded in
        wk_rows = np.concatenate(
            [wqkv[D + h * HD : D + (h + 1) * HD][perm] for h in heads], axis=0
        )
        wv_rows = np.concatenate(
            [wqkv[2 * D + h * HD : 2 * D + (h + 1) * HD] for h in heads], axis=0
        )
        wqk_l = np.concatenate([wq_rows, wk_rows], axis=0).T  # [D, 1024]
        wv_l = wv_rows.T  # [D, 512]
        din = np.concatenate([np.arange(h * HD, (h + 1) * HD) for h in heads])
        wo_l = wo[:, din].T  # [512, D]
        in_maps.append(
            {
                "xT": np.ascontiguousarray(x[b].T.astype(NP_CDT)),
                "wqk": np.ascontiguousarray(wqk_l.astype(NP_CDT)),
                "wv": np.ascontiguousarray(wv_l.astype(NP_CDT)),
                "wo": np.ascontiguousarray(wo_l.astype(NP_CDT)),
                "tabc": tabc,
                "tabs": tabs,
            }
        )
    return in_maps


def unshard_outputs(results):
    out = np.zeros((B, S, D), dtype=np.float32)
    for c in range(N_CORES):
        b = c // GROUPS
        out[b] += results[c]["out"].astype(np.float32)
    return out


_GRAPH_CACHE = {}


def kernel(x, freqs_cis, wqkv, wo):
    if "nc" not in _GRAPH_CACHE:
        _GRAPH_CACHE["nc"] = build_graph()
    nc = _GRAPH_CACHE["nc"]
    in_maps = shard_inputs(x, freqs_cis, wqkv, wo)
    res = run_bass_kernel_spmd(nc, in_maps, core_ids=list(range(N_CORES)))
    return unshard_outputs(res.results)


# revision 40
# speedup vs baseline: 1.0167x; 1.0024x over previous
"""Distributed attention kernel for 8 TRN2 NeuronCores.

Sharding: core c -> (batch b = c // 4, head-group g = c % 4).
Each core computes, for its batch element, 4 of the 16 heads end-to-end
(QKV projection, rotary, attention, output projection), producing a
partial output for the full [S, D] result. The host sums the 4 group
partials per batch element (the "all-reduce after wo" done at unshard).

All layouts are pre-arranged on the host so the device does zero
transposes:
  - xT    [D, S]   : x[b].T                       (rhs for qk / lhsT for v)
  - wqk   [D, 1024]: q,k weight rows (rotary-pair-permuted) transposed
  - wv    [D, 512] : v weight rows transposed
  - wo    [512, D] : wo columns for this group, transposed
  - tabc  [128, S] : cos table doubled across both partition halves
  - tabs  [128, S] : sin table doubled (1/sqrt(hd) folded into wq)

Rotary trick: q/k weight rows are permuted per head so dims [0:64] are
the even (real) rotary components and [64:128] the odd (imag) ones.
Then rotary is plain elementwise math on partition halves. Scores are
invariant to this permutation since q and k share it.

Attention is computed transposed (scores^T[j, i]) so the softmax
numerator AND attn@v need no transposes; the softmax denominator comes
from a ones-column matmul riding the same exp stream, and the division
is applied to the raw attn@v output.
"""

import numpy as np
import ml_dtypes

import concourse.tile as tile
from concourse import bacc, mybir
from concourse.bass_utils import run_bass_kernel_spmd

B, S, D = 2, 2048, 2048
NH, HD = 16, 128
N_CORES = 8
GROUPS = 4
LH = NH // GROUPS  # 4 local heads
EQK = 2 * LH * HD  # 1024 (q chunks then k chunks)
EV = LH * HD  # 512
P = 128
DC = D // P  # 16 contraction chunks over d
SC = S // P  # 16 chunks over s
F = 512  # matmul moving free dim (1 PSUM bank of f32)
NT = S // F  # 4

CDT = mybir.dt.bfloat16
NP_CDT = ml_dtypes.bfloat16
F32 = mybir.dt.float32
NP_OUT = NP_CDT  # device out dtype (partials; host upcasts + sums)


def _make_psum_rr(mm_pool, acc_pool, l_pool):
    def gen():
        seq = (
            ("mm", mm_pool),
            ("acc", acc_pool),
            ("mm", mm_pool),
            ("lsum", l_pool),
            ("acc", acc_pool),
            ("lsum", l_pool),
        )
        i = 0
        while True:
            tag, pool = seq[i % len(seq)]
            i += 1
            t1bank = pool.tile([P, F], F32, tag=tag, name=f"rr{i}")
            yield t1bank

    return gen()


def build_graph(num_devices: int = N_CORES, reps: int = 1):
    """reps > 1 replicates the whole computation (timing instrumentation)."""
    nc = bacc.Bacc(
        "TRN2", target_bir_lowering=False, debug=False, num_devices=num_devices
    )
    xT = nc.dram_tensor("xT", [D, S], CDT, kind="ExternalInput").ap()
    wqk = nc.dram_tensor("wqk", [D, EQK], CDT, kind="ExternalInput").ap()
    wv = nc.dram_tensor("wv", [D, EV], CDT, kind="ExternalInput").ap()
    wo = nc.dram_tensor("wo", [EV, D], CDT, kind="ExternalInput").ap()
    tabc = nc.dram_tensor("tabc", [P, S], CDT, kind="ExternalInput").ap()
    tabs = nc.dram_tensor("tabs", [P, S], CDT, kind="ExternalInput").ap()
    out = nc.dram_tensor("out", [S, D], CDT, kind="ExternalOutput").ap()

    xT_r = xT.rearrange("(c p) s -> p c s", p=P)  # [128, 16, 2048]
    wqk_r = wqk.rearrange("(c p) e -> p c e", p=P)  # [128, 16, 1024]
    wv_r = wv.rearrange("(c p) e -> p c e", p=P)  # [128, 16, 512]
    wo_r = wo.rearrange("(c p) o -> p c o", p=P)  # [128, 4, 2048]
    out_r = out.rearrange("(c p) o -> c p o", p=P)  # [16, 128, 2048]

    Exp = mybir.ActivationFunctionType.Exp
    sub = mybir.AluOpType.subtract

    with tile.TileContext(nc) as tc:
        with (
            tc.tile_pool(name="big", bufs=1) as big,  # x slot, reused for wo
            tc.tile_pool(name="wqkp", bufs=1) as wqkp,
            tc.tile_pool(name="wvp", bufs=1) as wvp,  # wv slot, reused for attn
            tc.tile_pool(name="data", bufs=1) as data,
            tc.tile_pool(name="tmp", bufs=1) as tmpp,
            tc.tile_pool(name="expp", bufs=3) as expp,
            tc.tile_pool(name="small", bufs=2) as small,
            tc.tile_pool(name="ostage", bufs=4) as ostagep,
            tc.tile_pool(name="mm", bufs=4, space="PSUM") as mm_pool,
            tc.tile_pool(name="acc", bufs=2, space="PSUM") as acc_pool,
            tc.tile_pool(name="lsum", bufs=2, space="PSUM") as l_pool,
        ):
          for _rep in range(reps):
            # Round-robin psum accumulators across all pools (the
            # attention-specific pools are idle outside attention) so the
            # PE can run several accumulation groups ahead of the consumers.
            rr = _make_psum_rr(mm_pool, acc_pool, l_pool)
            # ---------------- loads ----------------
            # Ordered so the first qk group's inputs land first: q-half of
            # wqk + the st=0 quarter of x (~4MB) instead of everything
            # (~14MB) before the PE can start.
            x_sb = big.tile([P, DC, S], CDT, tag="big")
            wqk_sb = wqkp.tile([P, DC, EQK], CDT, tag="wqk")
            wv_sb = wvp.tile([P, DC, EV], CDT, tag="wv")
            for c in range(DC):
                nc.sync.dma_start(wqk_sb[:, c, 0 : EQK // 2], wqk_r[:, c, 0 : EQK // 2])
            for c in range(DC):
                nc.sync.dma_start(x_sb[:, c, 0:F], xT_r[:, c, 0:F])
            tabc_sb = data.tile([P, S], CDT, tag="tabc")
            nc.sync.dma_start(tabc_sb[:], tabc)
            tabs_sb = data.tile([P, S], CDT, tag="tabs")
            nc.sync.dma_start(tabs_sb[:], tabs)
            for c in range(DC):
                nc.sync.dma_start(
                    wqk_sb[:, c, EQK // 2 : EQK], wqk_r[:, c, EQK // 2 : EQK]
                )
            for c in range(DC):
                nc.sync.dma_start(wv_sb[:, c, :], wv_r[:, c, :])
            for st in range(1, NT):
                for c in range(DC):
                    nc.sync.dma_start(
                        x_sb[:, c, st * F : (st + 1) * F],
                        xT_r[:, c, st * F : (st + 1) * F],
                    )

            rot_sb = data.tile([P, 2 * LH, S], CDT, tag="rot")
            v_sb = data.tile([P, SC, EV], CDT, tag="v")
            ones_sb = data.tile([P, P], CDT, tag="ones")
            nc.vector.memset(ones_sb[:], 1.0)

            # ---------------- qk projection + rotary ----------------
            # qkT[e, s] = sum_d wqk[d, e] * xT[d, s]; then rotary into rot_sb.
            # st-outer so the first groups only need the st=0 quarter of x.
            for st in range(NT):
                for ec in range(2 * LH):
                    ps = next(rr)
                    for c in range(DC):
                        nc.tensor.matmul(
                            ps[:],
                            lhsT=wqk_sb[:, c, ec * P : (ec + 1) * P],
                            rhs=x_sb[:, c, st * F : (st + 1) * F],
                            start=(c == 0),
                            stop=(c == DC - 1),
                        )
                    sl = slice(st * F, (st + 1) * F)
                    # Stage psum -> bf16 SBUF on the (idle) scalar engine:
                    # qs straight, qsw with partition halves swapped. DVE
                    # then runs aligned-base bf16-SBUF ops in the 2x mode.
                    # partitions 0:64 = even (re), 64:128 = odd (im)
                    qs = tmpp.tile([P, F], CDT, tag="qs")
                    nc.scalar.copy(out=qs[:], in_=ps[:])
                    qsw = tmpp.tile([P, F], CDT, tag="qsw")
                    nc.scalar.copy(out=qsw[0:64], in_=ps[64:128])
                    nc.scalar.copy(out=qsw[64:128], in_=ps[0:64])
                    t1 = tmpp.tile([P, F], CDT, tag="t1")
                    t2 = tmpp.tile([P, F], CDT, tag="t2")
                    nc.vector.tensor_mul(t1[:], qs[:], tabc_sb[:, sl])
                    nc.vector.tensor_mul(t2[:], qsw[:], tabs_sb[:, sl])
                    nc.vector.tensor_tensor(
                        rot_sb[0:64, ec, sl], t1[0:64], t2[0:64], sub
                    )
                    nc.vector.tensor_add(
                        rot_sb[64:128, ec, sl], t1[64:128], t2[64:128]
                    )

            # ---------------- v projection ----------------
            # v[s, e] = sum_d xT[d, s] * wv[d, e]
            for sc in range(SC):
                ps = next(rr)
                for c in range(DC):
                    nc.tensor.matmul(
                        ps[:],
                        lhsT=x_sb[:, c, sc * P : (sc + 1) * P],
                        rhs=wv_sb[:, c, :],
                        start=(c == 0),
                        stop=(c == DC - 1),
                    )
                nc.any.tensor_copy(out=v_sb[:, sc, :], in_=ps[:])

            attn_sb = wvp.tile([P, LH, S], CDT, tag="wv")  # reuses wv slot

            # ---------------- attention (transposed scores) ----------------
            for h in range(LH):
                for it in range(NT):
                    isl = slice(it * F, (it + 1) * F)
                    po = acc_pool.tile([P, F], F32, tag="acc")
                    pl = l_pool.tile([P, F], F32, tag="lsum")
                    for jc in range(SC):
                        ps = mm_pool.tile([P, F], F32, tag="mm")
                        # scores^T[j, i] = sum_hd k[hd, j] * q[hd, i]
                        nc.tensor.matmul(
                            ps[:],
                            lhsT=rot_sb[:, LH + h, jc * P : (jc + 1) * P],
                            rhs=rot_sb[:, h, isl],
                            start=True,
                            stop=True,
                        )
                        et = expp.tile([P, F], CDT, tag="exp")
                        nc.scalar.activation(out=et[:], in_=ps[:], func=Exp)
                        nc.tensor.matmul(
                            pl[:],
                            lhsT=ones_sb[:],
                            rhs=et[:],
                            start=(jc == 0),
                            stop=(jc == SC - 1),
                        )
                        nc.tensor.matmul(
                            po[:],
                            lhsT=v_sb[:, jc, h * P : (h + 1) * P],
                            rhs=et[:],
                            start=(jc == 0),
                            stop=(jc == SC - 1),
                        )
                    # pl rows are all equal (ones lhsT) -> reciprocal is
                    # already "broadcast" across partitions.
                    rl128 = small.tile([P, F], F32, tag="recip128")
                    nc.vector.reciprocal(rl128[:], pl[:])
                    nc.vector.tensor_mul(attn_sb[:, h, isl], po[:], rl128[:])

            # ---------------- output projection ----------------
            wo_sb = big.tile([P, LH, D], CDT, tag="big")  # reuses x slot
            for c in range(LH):
                nc.sync.dma_start(wo_sb[:, c, :], wo_r[:, c, :])

            for sc in range(SC):
                for ot in range(NT):
                    osl = slice(ot * F, (ot + 1) * F)
                    pw = next(rr)
                    for hc in range(LH):
                        nc.tensor.matmul(
                            pw[:],
                            lhsT=attn_sb[:, hc, sc * P : (sc + 1) * P],
                            rhs=wo_sb[:, hc, osl],
                            start=(hc == 0),
                            stop=(hc == LH - 1),
                        )
                    ost = ostagep.tile([P, F], CDT, tag="ostage")
                    nc.any.tensor_copy(out=ost[:], in_=pw[:])
                    nc.sync.dma_start(out_r[sc, :, osl], ost[:])

    nc.compile()
    return nc


def shard_inputs(x, freqs_cis, wqkv, wo):
    """Produce the 8 per-core input maps (host-side layout prep)."""
    x = np.asarray(x, dtype=np.float32)
    freqs_cis = np.asarray(freqs_cis, dtype=np.float32)
    wqkv = np.asarray(wqkv, dtype=np.float32)
    wo = np.asarray(wo, dtype=np.float32)

    perm = np.concatenate([np.arange(0, HD, 2), np.arange(1, HD, 2)])  # even|odd
    cos = freqs_cis[:, :, 0].T  # [64, S]
    sin = freqs_cis[:, :, 1].T
    scale = 1.0 / np.sqrt(HD)  # folded into wq rows below
    tabc = np.concatenate([cos, cos], axis=0)  # [128, S]
    tabs = np.concatenate([sin, sin], axis=0)

    tabc = np.ascontiguousarray(tabc.astype(NP_CDT))
    tabs = np.ascontiguousarray(tabs.astype(NP_CDT))

    in_maps = []
    for c in range(N_CORES):
        b, g = divmod(c, GROUPS)
        heads = range(g * LH, (g + 1) * LH)
        wq_rows = np.concatenate(
            [wqkv[h * HD : (h + 1) * HD][perm] for h in heads], axis=0
        ) * scale  # [512, D]; 1/sqrt(hd) folded in
        wk_rows = np.concatenate(
            [wqkv[D + h * HD : D + (h + 1) * HD][perm] for h in heads], axis=0
        )
        wv_rows = np.concatenate(
            [wqkv[2 * D + h * HD : 2 * D + (h + 1) * HD] for h in heads], axis=0
        )
        wqk_l = np.concatenate([wq_rows, wk_rows], axis=0).T  # [D, 1024]
        wv_l = wv_rows.T  # [D, 512]
        din = np.concatenate([np.arange(h * HD, (h + 1) * HD) for h in heads])
        wo_l = wo[:, din].T  # [512, D]
        in_maps.append(
            {
                "xT": np.ascontiguousarray(x[b].T.astype(NP_CDT)),
                "wqk": np.ascontiguousarray(wqk_l.astype(NP_CDT)),
                "wv": np.ascontiguousarray(wv_l.astype(NP_CDT)),
                "wo": np.ascontiguousarray(wo_l.astype(NP_CDT)),
                "tabc": tabc,
                "tabs": tabs,
            }
        )
    return in_maps


def unshard_outputs(results):
    out = np.zeros((B, S, D), dtype=np.float32)
    for c in range(N_CORES):
        b = c // GROUPS
        out[b] += results[c]["out"].astype(np.float32)
    return out


_GRAPH_CACHE = {}


def kernel(x, freqs_cis, wqkv, wo):
    if "nc" not in _GRAPH_CACHE:
        _GRAPH_CACHE["nc"] = build_graph()
    nc = _GRAPH_CACHE["nc"]
    in_maps = shard_inputs(x, freqs_cis, wqkv, wo)
    res = run_bass_kernel_spmd(nc, in_maps, core_ids=list(range(N_CORES)))
    return unshard_outputs(res.results)


# revision 45
# speedup vs baseline: 1.0686x; 1.0511x over previous
"""Distributed attention kernel for 8 TRN2 NeuronCores.

Sharding: core c -> (batch b = c // 4, head-group g = c % 4).
Each core computes, for its batch element, 4 of the 16 heads end-to-end
(QKV projection, rotary, attention, output projection), producing a
partial output for the full [S, D] result. The host sums the 4 group
partials per batch element (the "all-reduce after wo" done at unshard).

All layouts are pre-arranged on the host so the device does zero
transposes:
  - xT    [D, S]   : x[b].T                       (rhs for qk / lhsT for v)
  - wqk   [D, 1024]: q,k weight rows (rotary-pair-permuted) transposed
  - wv    [D, 512] : v weight rows transposed
  - wo    [512, D] : wo columns for this group, transposed
  - tabc  [128, S] : cos table doubled across both partition halves
  - tabs  [128, S] : sin table doubled (1/sqrt(hd) folded into wq)

Rotary trick: q/k weight rows are permuted per head so dims [0:64] are
the even (real) rotary components and [64:128] the odd (imag) ones.
Then rotary is plain elementwise math on partition halves. Scores are
invariant to this permutation since q and k share it.

Attention is computed transposed (scores^T[j, i]) so the softmax
numerator AND attn@v need no transposes; the softmax denominator comes
from a ones-column matmul riding the same exp stream, and the division
is applied to the raw attn@v output.
"""

import numpy as np
import ml_dtypes

import concourse.tile as tile
from concourse import bacc, mybir
from concourse.bass_utils import run_bass_kernel_spmd

B, S, D = 2, 2048, 2048
NH, HD = 16, 128
N_CORES = 8
GROUPS = 4
LH = NH // GROUPS  # 4 local heads
EQK = 2 * LH * HD  # 1024 (q chunks then k chunks)
EV = LH * HD  # 512
P = 128
DC = D // P  # 16 contraction chunks over d
SC = S // P  # 16 chunks over s
F = 512  # matmul moving free dim (1 PSUM bank of f32)
NT = S // F  # 4

CDT = mybir.dt.bfloat16
NP_CDT = ml_dtypes.bfloat16
F32 = mybir.dt.float32
NP_OUT = NP_CDT  # device out dtype (partials; host upcasts + sums)


def _make_psum_rr(mm_pool, acc_pool, l_pool):
    def gen():
        seq = (
            ("mm", mm_pool),
            ("acc", acc_pool),
            ("mm", mm_pool),
            ("lsum", l_pool),
            ("acc", acc_pool),
            ("lsum", l_pool),
        )
        i = 0
        while True:
            tag, pool = seq[i % len(seq)]
            i += 1
            t1bank = pool.tile([P, F], F32, tag=tag, name=f"rr{i}")
            yield t1bank

    return gen()


def build_graph(num_devices: int = N_CORES, reps: int = 1):
    """reps > 1 replicates the whole computation (timing instrumentation)."""
    nc = bacc.Bacc(
        "TRN2", target_bir_lowering=False, debug=False, num_devices=num_devices
    )
    xT = nc.dram_tensor("xT", [D, S], CDT, kind="ExternalInput").ap()
    wqk = nc.dram_tensor("wqk", [D, EQK], CDT, kind="ExternalInput").ap()
    wv = nc.dram_tensor("wv", [D, EV], CDT, kind="ExternalInput").ap()
    wo = nc.dram_tensor("wo", [EV, D], CDT, kind="ExternalInput").ap()
    tabc = nc.dram_tensor("tabc", [P, S], CDT, kind="ExternalInput").ap()
    tabs = nc.dram_tensor("tabs", [P, S], CDT, kind="ExternalInput").ap()
    out = nc.dram_tensor("out", [S, D], CDT, kind="ExternalOutput").ap()

    xT_r = xT.rearrange("(c p) s -> p c s", p=P)  # [128, 16, 2048]
    wqk_r = wqk.rearrange("(c p) e -> p c e", p=P)  # [128, 16, 1024]
    wv_r = wv.rearrange("(c p) e -> p c e", p=P)  # [128, 16, 512]
    wo_r = wo.rearrange("(c p) o -> p c o", p=P)  # [128, 4, 2048]
    out_r = out.rearrange("(c p) o -> c p o", p=P)  # [16, 128, 2048]

    Exp = mybir.ActivationFunctionType.Exp
    sub = mybir.AluOpType.subtract

    with tile.TileContext(nc) as tc:
        with (
            tc.tile_pool(name="big", bufs=1) as big,  # x slot, reused for wo
            tc.tile_pool(name="wqkp", bufs=1) as wqkp,
            tc.tile_pool(name="wvp", bufs=1) as wvp,  # wv slot, reused for attn
            tc.tile_pool(name="data", bufs=1) as data,
            tc.tile_pool(name="tmp", bufs=1) as tmpp,
            tc.tile_pool(name="expp", bufs=3) as expp,
            tc.tile_pool(name="small", bufs=2) as small,
            tc.tile_pool(name="ostage", bufs=4) as ostagep,
            tc.tile_pool(name="mm", bufs=4, space="PSUM") as mm_pool,
            tc.tile_pool(name="acc", bufs=2, space="PSUM") as acc_pool,
            tc.tile_pool(name="lsum", bufs=2, space="PSUM") as l_pool,
        ):
          for _rep in range(reps):
            # Round-robin psum accumulators across all pools (the
            # attention-specific pools are idle outside attention) so the
            # PE can run several accumulation groups ahead of the consumers.
            rr = _make_psum_rr(mm_pool, acc_pool, l_pool)
            # ---------------- loads ----------------
            # Ordered so the first qk group's inputs land first: q-half of
            # wqk + the st=0 quarter of x (~4MB) instead of everything
            # (~14MB) before the PE can start.
            x_sb = big.tile([P, DC, S], CDT, tag="big")
            wqk_sb = wqkp.tile([P, DC, EQK], CDT, tag="wqk")
            wv_sb = wvp.tile([P, DC, EV], CDT, tag="wv")
            for c in range(DC):
                nc.sync.dma_start(wqk_sb[:, c, 0 : EQK // 2], wqk_r[:, c, 0 : EQK // 2])
            for c in range(DC):
                nc.sync.dma_start(x_sb[:, c, 0:F], xT_r[:, c, 0:F])
            tabc_sb = data.tile([P, S], CDT, tag="tabc")
            nc.sync.dma_start(tabc_sb[:], tabc)
            tabs_sb = data.tile([P, S], CDT, tag="tabs")
            nc.sync.dma_start(tabs_sb[:], tabs)
            for c in range(DC):
                nc.sync.dma_start(
                    wqk_sb[:, c, EQK // 2 : EQK], wqk_r[:, c, EQK // 2 : EQK]
                )
            for c in range(DC):
                nc.sync.dma_start(wv_sb[:, c, :], wv_r[:, c, :])
            for st in range(1, NT):
                for c in range(DC):
                    nc.sync.dma_start(
                        x_sb[:, c, st * F : (st + 1) * F],
                        xT_r[:, c, st * F : (st + 1) * F],
                    )

            rot_sb = data.tile([P, 2 * LH, S], CDT, tag="rot")
            v_sb = data.tile([P, SC, EV], CDT, tag="v")
            # f32 ones; bitcast to float32r at the reduce matmul
            # (1 cyc/row at N=512, ~1e-4 matmul precision)
            ones_f32 = data.tile([P, P], F32, tag="ones32")
            nc.vector.memset(ones_f32[:], 1.0)
            ones_fr = data.tile([P, P], mybir.dt.float32r, tag="ones")
            nc.vector.tensor_copy(out=ones_fr[:], in_=ones_f32[:])

            # ---------------- qk projection + rotary ----------------
            # qkT[e, s] = sum_d wqk[d, e] * xT[d, s]; then rotary into rot_sb.
            # st-outer so the first groups only need the st=0 quarter of x.
            for st in range(NT):
                for ec in range(2 * LH):
                    ps = next(rr)
                    for c in range(DC):
                        nc.tensor.matmul(
                            ps[:],
                            lhsT=wqk_sb[:, c, ec * P : (ec + 1) * P],
                            rhs=x_sb[:, c, st * F : (st + 1) * F],
                            start=(c == 0),
                            stop=(c == DC - 1),
                        )
                    sl = slice(st * F, (st + 1) * F)
                    # Stage psum -> bf16 SBUF on the (idle) scalar engine:
                    # qs straight, qsw with partition halves swapped. DVE
                    # then runs aligned-base bf16-SBUF ops in the 2x mode.
                    # partitions 0:64 = even (re), 64:128 = odd (im)
                    qs = tmpp.tile([P, F], CDT, tag="qs")
                    nc.scalar.copy(out=qs[:], in_=ps[:])
                    qsw = tmpp.tile([P, F], CDT, tag="qsw")
                    nc.scalar.copy(out=qsw[0:64], in_=ps[64:128])
                    nc.scalar.copy(out=qsw[64:128], in_=ps[0:64])
                    t1 = tmpp.tile([P, F], CDT, tag="t1")
                    t2 = tmpp.tile([P, F], CDT, tag="t2")
                    nc.vector.tensor_mul(t1[:], qs[:], tabc_sb[:, sl])
                    nc.vector.tensor_mul(t2[:], qsw[:], tabs_sb[:, sl])
                    nc.vector.tensor_tensor(
                        rot_sb[0:64, ec, sl], t1[0:64], t2[0:64], sub
                    )
                    nc.vector.tensor_add(
                        rot_sb[64:128, ec, sl], t1[64:128], t2[64:128]
                    )

            # ---------------- v projection ----------------
            # v[s, e] = sum_d xT[d, s] * wv[d, e]
            for sc in range(SC):
                ps = next(rr)
                for c in range(DC):
                    nc.tensor.matmul(
                        ps[:],
                        lhsT=x_sb[:, c, sc * P : (sc + 1) * P],
                        rhs=wv_sb[:, c, :],
                        start=(c == 0),
                        stop=(c == DC - 1),
                    )
                nc.any.tensor_copy(out=v_sb[:, sc, :], in_=ps[:])

            attn_sb = wvp.tile([P, LH, S], CDT, tag="wv")  # reuses wv slot

            # ---------------- attention (transposed scores) ----------------
            # The softmax denominator comes from an elementwise sum of the
            # exp tiles on the DVE (bf16 pair adds in the 2x mode, f32
            # combines) + one f32r ones-matmul per tile for the final
            # cross-partition reduce. This removes the per-chunk PE
            # ones-matmul, dropping the inner-loop cadence from 3 to 2
            # matmuls per chunk (the ACT exp becomes the pacer).
            for it in range(NT):
                for h in range(LH):
                    isl = slice(it * F, (it + 1) * F)
                    po = acc_pool.tile([P, F], F32, tag="acc")
                    accl = small.tile([P, F], mybir.dt.float32r, tag="accl")
                    for jj in range(SC // 2):
                        ets = []
                        for k in range(2):
                            jc = 2 * jj + k
                            ps = mm_pool.tile([P, F], F32, tag="mm")
                            # scores^T[j, i] = sum_hd k[hd, j] * q[hd, i]
                            nc.tensor.matmul(
                                ps[:],
                                lhsT=rot_sb[:, LH + h, jc * P : (jc + 1) * P],
                                rhs=rot_sb[:, h, isl],
                                start=True,
                                stop=True,
                            )
                            et = expp.tile([P, F], CDT, tag="exp")
                            nc.scalar.activation(out=et[:], in_=ps[:], func=Exp)
                            nc.tensor.matmul(
                                po[:],
                                lhsT=v_sb[:, jc, h * P : (h + 1) * P],
                                rhs=et[:],
                                start=(jc == 0),
                                stop=(jc == SC - 1),
                            )
                            ets.append(et)
                        if jj == 0:
                            nc.vector.tensor_add(accl[:], ets[0][:], ets[1][:])
                        else:
                            pr = expp.tile([P, F], CDT, tag="pair")
                            nc.vector.tensor_add(pr[:], ets[0][:], ets[1][:])
                            nc.vector.tensor_add(accl[:], accl[:], pr[:])
                    pl = l_pool.tile([P, F], F32, tag="lsum")
                    nc.tensor.matmul(
                        pl[:],
                        lhsT=ones_fr[:],
                        rhs=accl[:],
                        start=True,
                        stop=True,
                    )
                    # pl rows are all equal (ones lhsT) -> reciprocal is
                    # already "broadcast" across partitions.
                    rl128 = small.tile([P, F], F32, tag="recip128")
                    nc.vector.reciprocal(rl128[:], pl[:])
                    nc.vector.tensor_mul(attn_sb[:, h, isl], po[:], rl128[:])

            # ---------------- output projection ----------------
            wo_sb = big.tile([P, LH, D], CDT, tag="big")  # reuses x slot
            for c in range(LH):
                nc.sync.dma_start(wo_sb[:, c, :], wo_r[:, c, :])

            for sc in range(SC):
                for ot in range(NT):
                    osl = slice(ot * F, (ot + 1) * F)
                    pw = next(rr)
                    for hc in range(LH):
                        nc.tensor.matmul(
                            pw[:],
                            lhsT=attn_sb[:, hc, sc * P : (sc + 1) * P],
                            rhs=wo_sb[:, hc, osl],
                            start=(hc == 0),
                            stop=(hc == LH - 1),
                        )
                    ost = ostagep.tile([P, F], CDT, tag="ostage")
                    nc.any.tensor_copy(out=ost[:], in_=pw[:])
                    nc.sync.dma_start(out_r[sc, :, osl], ost[:])

    nc.compile()
    return nc


def shard_inputs(x, freqs_cis, wqkv, wo):
    """Produce the 8 per-core input maps (host-side layout prep)."""
    x = np.asarray(x, dtype=np.float32)
    freqs_cis = np.asarray(freqs_cis, dtype=np.float32)
    wqkv = np.asarray(wqkv, dtype=np.float32)
    wo = np.asarray(wo, dtype=np.float32)

    perm = np.concatenate([np.arange(0, HD, 2), np.arange(1, HD, 2)])  # even|odd
    cos = freqs_cis[:, :, 0].T  # [64, S]
    sin = freqs_cis[:, :, 1].T
    scale = 1.0 / np.sqrt(HD)  # folded into wq rows below
    tabc = np.concatenate([cos, cos], axis=0)  # [128, S]
    tabs = np.concatenate([sin, sin], axis=0)

    tabc = np.ascontiguousarray(tabc.astype(NP_CDT))
    tabs = np.ascontiguousarray(tabs.astype(NP_CDT))

    in_maps = []
    for c in range(N_CORES):
        b, g = divmod(c, GROUPS)
        heads = range(g * LH, (g + 1) * LH)
        wq_rows = np.concatenate(
            [wqkv[h * HD : (h + 1) * HD][perm] for h in heads], axis=0
        ) * scale  # [512, D]; 1/sqrt(hd) folded in
        wk_rows = np.concatenate(
            [wqkv[D + h * HD : D + (h + 1) * HD][perm] for h in heads], axis=0
        )
        wv_rows = np.concatenate(
            [wqkv[2 * D + h * HD : 2 * D + (h + 1) * HD] for h in heads], axis=0
        )
        wqk_l = np.concatenate([wq_rows, wk_rows], axis=0).T  # [D, 1024]
        wv_l = wv_rows.T  # [D, 512]
        din = np.concatenate([np.arange(h * HD, (h + 1) * HD) for h in heads])
        wo_l = wo[:, din].T  # [512, D]
        in_maps.append(
            {
                "xT": np.ascontiguousarray(x[b].T.astype(NP_CDT)),
                "wqk": np.ascontiguousarray(wqk_l.astype(NP_CDT)),
                "wv": np.ascontiguousarray(wv_l.astype(NP_CDT)),
                "wo": np.ascontiguousarray(wo_l.astype(NP_CDT)),
                "tabc": tabc,
                "tabs": tabs,
            }
        )
    return in_maps


def unshard_outputs(results):
    out = np.zeros((B, S, D), dtype=np.float32)
    for c in range(N_CORES):
        b = c // GROUPS
        out[b] += results[c]["out"].astype(np.float32)
    return out


_GRAPH_CACHE = {}


def kernel(x, freqs_cis, wqkv, wo):
    if "nc" not in _GRAPH_CACHE:
        _GRAPH_CACHE["nc"] = build_graph()
    nc = _GRAPH_CACHE["nc"]
    in_maps = shard_inputs(x, freqs_cis, wqkv, wo)
    res = run_bass_kernel_spmd(nc, in_maps, core_ids=list(range(N_CORES)))
    return unshard_outputs(res.results)


# revision 48
# speedup vs baseline: 1.2151x; 1.1371x over previous
"""Distributed attention kernel for 8 TRN2 NeuronCores.

Sharding: core c -> (batch b = c // 4, head-group g = c % 4).
Each core computes, for its batch element, 4 of the 16 heads end-to-end
(QKV projection, rotary, attention, output projection), producing a
partial output for the full [S, D] result. The host sums the 4 group
partials per batch element (the "all-reduce after wo" done at unshard).

All layouts are pre-arranged on the host so the device does zero
transposes:
  - xT    [D, S]   : x[b].T                       (rhs for qk / lhsT for v)
  - wqk   [D, 1024]: q,k weight rows (rotary-pair-permuted) transposed
  - wv    [D, 512] : v weight rows transposed
  - wo    [512, D] : wo columns for this group, transposed
  - tabc  [128, S] : cos table doubled across both partition halves
  - tabs  [128, S] : sin table doubled (1/sqrt(hd) folded into wq)

Rotary trick: q/k weight rows are permuted per head so dims [0:64] are
the even (real) rotary components and [64:128] the odd (imag) ones.
Then rotary is plain elementwise math on partition halves. Scores are
invariant to this permutation since q and k share it.

Attention is computed transposed (scores^T[j, i]) so the softmax
numerator AND attn@v need no transposes. The softmax denominator is an
elementwise sum of the exp tiles on the DVE (bf16 pair adds in the 2x
mode + f32r combines) finished by one float32r ones-matmul per i-tile
(cross-partition reduce whose psum rows all equal l -- a free partition
broadcast); the division is applied to the raw attn@v output.
"""

import numpy as np
import ml_dtypes

import concourse.tile as tile
from concourse import bacc, mybir
from concourse.bass_utils import run_bass_kernel_spmd

B, S, D = 2, 2048, 2048
NH, HD = 16, 128
N_CORES = 8
GROUPS = 4
LH = NH // GROUPS  # 4 local heads
EQK = 2 * LH * HD  # 1024 (q chunks then k chunks)
EV = LH * HD  # 512
P = 128
DC = D // P  # 16 contraction chunks over d
SC = S // P  # 16 chunks over s
F = 512  # matmul moving free dim (1 PSUM bank of f32)
NT = S // F  # 4

CDT = mybir.dt.bfloat16
NP_CDT = ml_dtypes.bfloat16
F32 = mybir.dt.float32
NP_OUT = NP_CDT  # device out dtype (partials; host upcasts + sums)


def _make_psum_rr(mm_pool, acc_pool, l_pool):
    def gen():
        seq = (
            ("mm", mm_pool),
            ("acc", acc_pool),
            ("mm", mm_pool),
            ("lsum", l_pool),
            ("acc", acc_pool),
            ("lsum", l_pool),
        )
        i = 0
        while True:
            tag, pool = seq[i % len(seq)]
            i += 1
            t1bank = pool.tile([P, F], F32, tag=tag, name=f"rr{i}")
            yield t1bank

    return gen()


def build_graph(num_devices: int = N_CORES, reps: int = 1):
    """reps > 1 replicates the whole computation (timing instrumentation)."""
    nc = bacc.Bacc(
        "TRN2", target_bir_lowering=False, debug=False, num_devices=num_devices
    )
    xT = nc.dram_tensor("xT", [D, S], CDT, kind="ExternalInput").ap()
    wqk = nc.dram_tensor("wqk", [D, EQK], CDT, kind="ExternalInput").ap()
    wv = nc.dram_tensor("wv", [D, EV], CDT, kind="ExternalInput").ap()
    wo = nc.dram_tensor("wo", [EV, D], CDT, kind="ExternalInput").ap()
    tabc = nc.dram_tensor("tabc", [P, S], CDT, kind="ExternalInput").ap()
    tabs = nc.dram_tensor("tabs", [P, S], CDT, kind="ExternalInput").ap()
    out = nc.dram_tensor("out", [S, D], CDT, kind="ExternalOutput").ap()

    xT_r = xT.rearrange("(c p) s -> p c s", p=P)  # [128, 16, 2048]
    wqk_r = wqk.rearrange("(c p) e -> p c e", p=P)  # [128, 16, 1024]
    wv_r = wv.rearrange("(c p) e -> p c e", p=P)  # [128, 16, 512]
    wo_r = wo.rearrange("(c p) o -> p c o", p=P)  # [128, 4, 2048]
    out_r = out.rearrange("(c p) o -> c p o", p=P)  # [16, 128, 2048]

    Exp = mybir.ActivationFunctionType.Exp
    sub = mybir.AluOpType.subtract

    with tile.TileContext(nc) as tc:
        with (
            tc.tile_pool(name="big", bufs=1) as big,  # x slot, reused for wo
            tc.tile_pool(name="wqkp", bufs=1) as wqkp,
            tc.tile_pool(name="wvp", bufs=1) as wvp,  # wv slot, reused for attn
            tc.tile_pool(name="data", bufs=1) as data,
            tc.tile_pool(name="tmp", bufs=1) as tmpp,
            tc.tile_pool(name="expp", bufs=3) as expp,
            tc.tile_pool(name="small", bufs=2) as small,
            tc.tile_pool(name="ostage", bufs=6) as ostagep,
            tc.tile_pool(name="mm", bufs=4, space="PSUM") as mm_pool,
            tc.tile_pool(name="acc", bufs=2, space="PSUM") as acc_pool,
            tc.tile_pool(name="lsum", bufs=2, space="PSUM") as l_pool,
        ):
          for _rep in range(reps):
            # Round-robin psum accumulators across all pools (the
            # attention-specific pools are idle outside attention) so the
            # PE can run several accumulation groups ahead of the consumers.
            rr = _make_psum_rr(mm_pool, acc_pool, l_pool)
            # ---------------- loads ----------------
            # Ordered so the first qk group's inputs land first: q-half of
            # wqk + the st=0 quarter of x (~4MB) instead of everything
            # (~14MB) before the PE can start.
            x_sb = big.tile([P, DC, S], CDT, tag="big")
            wqk_sb = wqkp.tile([P, DC, EQK], CDT, tag="wqk")
            wv_sb = wvp.tile([P, DC, EV], CDT, tag="wv")
            for c in range(DC):
                nc.sync.dma_start(wqk_sb[:, c, 0 : EQK // 2], wqk_r[:, c, 0 : EQK // 2])
            for c in range(DC):
                nc.sync.dma_start(x_sb[:, c, 0:F], xT_r[:, c, 0:F])
            tabc_sb = data.tile([P, S], CDT, tag="tabc")
            nc.sync.dma_start(tabc_sb[:], tabc)
            tabs_sb = data.tile([P, S], CDT, tag="tabs")
            nc.sync.dma_start(tabs_sb[:], tabs)
            for c in range(DC):
                nc.sync.dma_start(
                    wqk_sb[:, c, EQK // 2 : EQK], wqk_r[:, c, EQK // 2 : EQK]
                )
            for c in range(DC):
                nc.sync.dma_start(wv_sb[:, c, :], wv_r[:, c, :])
            for st in range(1, NT):
                for c in range(DC):
                    nc.sync.dma_start(
                        x_sb[:, c, st * F : (st + 1) * F],
                        xT_r[:, c, st * F : (st + 1) * F],
                    )

            rot_sb = data.tile([P, 2 * LH, S], CDT, tag="rot")
            v_sb = data.tile([P, SC, EV], CDT, tag="v")
            # f32 ones; bitcast to float32r at the reduce matmul
            # (1 cyc/row at N=512, ~1e-4 matmul precision)
            ones_f32 = data.tile([P, P], F32, tag="ones32")
            nc.vector.memset(ones_f32[:], 1.0)
            ones_fr = data.tile([P, P], mybir.dt.float32r, tag="ones")
            nc.vector.tensor_copy(out=ones_fr[:], in_=ones_f32[:])

            # ---------------- qk projection + rotary ----------------
            # qkT[e, s] = sum_d wqk[d, e] * xT[d, s]; then rotary into rot_sb.
            # st-outer so the first groups only need the st=0 quarter of x.
            for st in range(NT):
                for ec in range(2 * LH):
                    ps = next(rr)
                    for c in range(DC):
                        nc.tensor.matmul(
                            ps[:],
                            lhsT=wqk_sb[:, c, ec * P : (ec + 1) * P],
                            rhs=x_sb[:, c, st * F : (st + 1) * F],
                            start=(c == 0),
                            stop=(c == DC - 1),
                        )
                    sl = slice(st * F, (st + 1) * F)
                    # Stage psum -> bf16 SBUF on the (idle) scalar engine:
                    # qs straight, qsw with partition halves swapped. DVE
                    # then runs aligned-base bf16-SBUF ops in the 2x mode.
                    # partitions 0:64 = even (re), 64:128 = odd (im)
                    qs = tmpp.tile([P, F], CDT, tag="qs")
                    nc.scalar.copy(out=qs[:], in_=ps[:])
                    qsw = tmpp.tile([P, F], CDT, tag="qsw")
                    nc.scalar.copy(out=qsw[0:64], in_=ps[64:128])
                    nc.scalar.copy(out=qsw[64:128], in_=ps[0:64])
                    t1 = tmpp.tile([P, F], CDT, tag="t1")
                    t2 = tmpp.tile([P, F], CDT, tag="t2")
                    nc.vector.tensor_mul(t1[:], qs[:], tabc_sb[:, sl])
                    nc.vector.tensor_mul(t2[:], qsw[:], tabs_sb[:, sl])
                    nc.vector.tensor_tensor(
                        rot_sb[0:64, ec, sl], t1[0:64], t2[0:64], sub
                    )
                    nc.vector.tensor_add(
                        rot_sb[64:128, ec, sl], t1[64:128], t2[64:128]
                    )

            # ---------------- v projection ----------------
            # v[s, e] = sum_d xT[d, s] * wv[d, e]
            for sc in range(SC):
                ps = next(rr)
                for c in range(DC):
                    nc.tensor.matmul(
                        ps[:],
                        lhsT=x_sb[:, c, sc * P : (sc + 1) * P],
                        rhs=wv_sb[:, c, :],
                        start=(c == 0),
                        stop=(c == DC - 1),
                    )
                nc.any.tensor_copy(out=v_sb[:, sc, :], in_=ps[:])

            attn_sb = wvp.tile([P, LH, S], CDT, tag="wv")  # reuses wv slot

            # ---------------- attention (transposed scores) ----------------
            # The softmax denominator comes from an elementwise sum of the
            # exp tiles on the DVE (bf16 pair adds in the 2x mode, f32
            # combines) + one f32r ones-matmul per tile for the final
            # cross-partition reduce. This removes the per-chunk PE
            # ones-matmul, dropping the inner-loop cadence from 3 to 2
            # matmuls per chunk (the ACT exp becomes the pacer).
            for it in range(NT):
                for h in range(LH):
                    isl = slice(it * F, (it + 1) * F)
                    po = acc_pool.tile([P, F], F32, tag="acc")
                    accl = small.tile([P, F], mybir.dt.float32r, tag="accl")
                    for jj in range(SC // 2):
                        ets = []
                        for k in range(2):
                            jc = 2 * jj + k
                            ps = mm_pool.tile([P, F], F32, tag="mm")
                            # scores^T[j, i] = sum_hd k[hd, j] * q[hd, i]
                            nc.tensor.matmul(
                                ps[:],
                                lhsT=rot_sb[:, LH + h, jc * P : (jc + 1) * P],
                                rhs=rot_sb[:, h, isl],
                                start=True,
                                stop=True,
                            )
                            et = expp.tile([P, F], CDT, tag="exp")
                            nc.scalar.activation(out=et[:], in_=ps[:], func=Exp)
                            nc.tensor.matmul(
                                po[:],
                                lhsT=v_sb[:, jc, h * P : (h + 1) * P],
                                rhs=et[:],
                                start=(jc == 0),
                                stop=(jc == SC - 1),
                            )
                            ets.append(et)
                        if jj == 0:
                            nc.vector.tensor_add(accl[:], ets[0][:], ets[1][:])
                        else:
                            pr = expp.tile([P, F], CDT, tag="pair")
                            nc.vector.tensor_add(pr[:], ets[0][:], ets[1][:])
                            nc.vector.tensor_add(accl[:], accl[:], pr[:])
                    pl = l_pool.tile([P, F], F32, tag="lsum")
                    nc.tensor.matmul(
                        pl[:],
                        lhsT=ones_fr[:],
                        rhs=accl[:],
                        start=True,
                        stop=True,
                    )
                    # pl rows are all equal (ones lhsT) -> reciprocal is
                    # already "broadcast" across partitions.
                    rl128 = small.tile([P, F], F32, tag="recip128")
                    nc.vector.reciprocal_approx_fast(rl128[:], pl[:])
                    nc.vector.tensor_mul(attn_sb[:, h, isl], po[:], rl128[:])

            # ---------------- output projection ----------------
            wo_sb = big.tile([P, LH, D], CDT, tag="big")  # reuses x slot
            for c in range(LH):
                nc.sync.dma_start(wo_sb[:, c, :], wo_r[:, c, :])

            for sc in range(SC):
                for ot in range(NT):
                    osl = slice(ot * F, (ot + 1) * F)
                    pw = mm_pool.tile([P, F], F32, tag="mm")
                    for hc in range(LH):
                        nc.tensor.matmul(
                            pw[:],
                            lhsT=attn_sb[:, hc, sc * P : (sc + 1) * P],
                            rhs=wo_sb[:, hc, osl],
                            start=(hc == 0),
                            stop=(hc == LH - 1),
                        )
                    ost = ostagep.tile([P, F], CDT, tag="ostage")
                    nc.any.tensor_copy(out=ost[:], in_=pw[:])
                    nc.sync.dma_start(out_r[sc, :, osl], ost[:])

    nc.compile()
    return nc


def shard_inputs(x, freqs_cis, wqkv, wo):
    """Produce the 8 per-core input maps (host-side layout prep)."""
    x = np.asarray(x, dtype=np.float32)
    freqs_cis = np.asarray(freqs_cis, dtype=np.float32)
    wqkv = np.asarray(wqkv, dtype=np.float32)
    wo = np.asarray(wo, dtype=np.float32)

    perm = np.concatenate([np.arange(0, HD, 2), np.arange(1, HD, 2)])  # even|odd
    cos = freqs_cis[:, :, 0].T  # [64, S]
    sin = freqs_cis[:, :, 1].T
    scale = 1.0 / np.sqrt(HD)  # folded into wq rows below
    tabc = np.concatenate([cos, cos], axis=0)  # [128, S]
    tabs = np.concatenate([sin, sin], axis=0)

    tabc = np.ascontiguousarray(tabc.astype(NP_CDT))
    tabs = np.ascontiguousarray(tabs.astype(NP_CDT))

    in_maps = []
    for c in range(N_CORES):
        b, g = divmod(c, GROUPS)
        heads = range(g * LH, (g + 1) * LH)
        wq_rows = np.concatenate(
            [wqkv[h * HD : (h + 1) * HD][perm] for h in heads], axis=0
        ) * scale  # [512, D]; 1/sqrt(hd) folded in
        wk_rows = np.concatenate(
            [wqkv[D + h * HD : D + (h + 1) * HD][perm] for h in heads], axis=0
        )
        wv_rows = np.concatenate(
            [wqkv[2 * D + h * HD : 2 * D + (h + 1) * HD] for h in heads], axis=0
        )
        wqk_l = np.concatenate([wq_rows, wk_rows], axis=0).T  # [D, 1024]
        wv_l = wv_rows.T  # [D, 512]
        din = np.concatenate([np.arange(h * HD, (h + 1) * HD) for h in heads])
        wo_l = wo[:, din].T  # [512, D]
        in_maps.append(
            {
                "xT": np.ascontiguousarray(x[b].T.astype(NP_CDT)),
                "wqk": np.ascontiguousarray(wqk_l.astype(NP_CDT)),
                "wv": np.ascontiguousarray(wv_l.astype(NP_CDT)),
                "wo": np.ascontiguousarray(wo_l.astype(NP_CDT)),
                "tabc": tabc,
                "tabs": tabs,
            }
        )
    return in_maps


def unshard_outputs(results):
    out = np.zeros((B, S, D), dtype=np.float32)
    for c in range(N_CORES):
        b = c // GROUPS
        out[b] += results[c]["out"].astype(np.float32)
    return out


_GRAPH_CACHE = {}


def kernel(x, freqs_cis, wqkv, wo):
    if "nc" not in _GRAPH_CACHE:
        _GRAPH_CACHE["nc"] = build_graph()
    nc = _GRAPH_CACHE["nc"]
    in_maps = shard_inputs(x, freqs_cis, wqkv, wo)
    res = run_bass_kernel_spmd(nc, in_maps, core_ids=list(range(N_CORES)))
    return unshard_outputs(res.results)
